# revision 37
# baseline (speedup 1.0000x reference)
"""Bidirectional DSS/Mamba block on 8 trn2 cores (Bass/Tile) — v8.

Sharding: core = (batch b = core//2, d_inner half = core%2).

HW-measured basis: the DVE scan runs ~2.25ns/elem regardless of dtype
(recurrence feedback latency), so the 64 [128,900]-equivalent scans are a
~125us floor; everything else is arranged to saturate the DVE from ~30us
to the end of the scan phase and to shrink the exchange + tail latency.
Measured facts driving this version (vs the 266us v2 baseline):
- DVE tensor_tensor has a ~140-360ns fixed overhead: groups 1-3 run ONE
  merged scan per (dir, dtile) across all 4 segments (the zeroed 4-column
  pads between SEG=904 segments reset the state in both directions), one
  merged h*=C multiply, and one merged dbu multiply (w2 repeated via a
  stride-0 AP dim). Validated bit-exact against numpy on hardware.
- One activation table is active at a time (1.28us per switch); blanking
  the exp-only/ln-only sets steers Exp AND Ln onto
  natural_log_exp_and_others so the dt chain exp->ln->dA-exps runs with a
  single load (verified: no reload between Ln and the dA exps).
- dt = ln(exp(q + b_dt) + 1): the +1 rides the Ln bias port.
- The [128, 4*900] broadcast DMA takes ~9us to land (921KB); group 0's
  B and C rows are fetched as per-segment DMAs and its dbu multiplies and
  scans are hoisted into the startup section interleaved per segment
  [dbu_k, scan_fwd_k, scan_bwd_k] — otherwise the DVE's in-order queue
  head-of-line blocks on broadcast slices that haven't landed.
- z in-proj sits between x_proj and the dt matmuls: its Silu runs under
  the still-loaded initial Silu table (a mid-scan-loop placement caused
  extra table loads; the tile scheduler reorders dependency-free code).
- The yg multiply + pooled-sum accumulate run inside the scan loop as
  each (dir, dtile) PSUM accumulator completes.
- The pooled exchange is fp32 (collective time is latency-noise, not
  bytes); the gate matvec accumulates its bias via a ones-row matmul, the
  sigmoid runs on ACT with its table prefetch pinned to m_sb (a
  dependency-free op would be scheduler-hoisted into the scan phase), and
  the gate scales the W_out rows ([128,256] per-partition muls) instead
  of the yg tiles.
- No DVE-paced "warm" chain: the tile scheduler hoists dependency-free
  work, so such a chain steals mid-scan DVE cycles (measured +9us).
"""

import sys

sys.path.insert(0, "/opt/trn_rl_repo")

from contextlib import ExitStack

import ml_dtypes
import numpy as np

import concourse.bass as bass
import concourse.bacc as bacc
import concourse.tile as tile
from concourse import mybir
from concourse.bass_utils import run_bass_kernel_spmd
from concourse.hw_specs import get_activation_tables

F32 = mybir.dt.float32
BF16 = mybir.dt.bfloat16
AF = mybir.ActivationFunctionType
OP = mybir.AluOpType

B, L, DM, DS, DI, R = 4, 900, 256, 16, 512, 16
DH = DI // 2          # d_inner channels per core
NDT = DH // 128       # 128-channel tiles per core (2)
FCH = [(0, 512), (512, L - 512)]  # PSUM-bank-aligned L chunks
SEG = 904             # aligned segment stride in grouped tiles (16B)


class _Bacc(bacc.Bacc):
    """Bacc that steers Exp and Ln onto the combined exp+ln table set.

    First-fit selection would pick exp_and_others / natural_log and
    thrash the single active table across the dt chain. Blanking those
    sets (positions preserved, so the emitted act_func_set_id still
    indexes act_info.json) makes both functions resolve to
    natural_log_exp_and_others.
    """

    def insert_act_table_loads(self):
        has_activation = any(
            isinstance(i, mybir.InstActivation)
            for b in self.main_func.blocks
            for i in b.instructions
        )
        if not has_activation:
            return
        drop = {"exp_and_others", "natural_log"}
        tables = [
            (n, (set() if n in drop else s))
            for n, s in get_activation_tables(self.m.arch).items()
        ]
        bacc._bass_rust.insert_act_table_loads(self, tables)


def _seg(t, k, f0=0, fl=L):
    """Slice segment k's [f0:f0+fl] columns of a grouped [128, G*SEG] tile."""
    return t[:, k * SEG + f0 : k * SEG + f0 + fl]


def _build_module(shared_a: bool):
    G = 4 if shared_a else 2      # n-group size (SBUF bound when A_f != A_b)
    NG = DS // G
    GW = G * SEG                  # grouped tile width

    nc = _Bacc("TRN2", num_devices=8)

    ein = lambda n, s, d=F32: nc.dram_tensor(n, s, d, kind="ExternalInput")
    hsT = ein("hsT", [DM, L], BF16)
    WinxT = ein("WinxT", [DM, DI], BF16)
    WinzT = ein("WinzT", [DM, DH], BF16)
    WxT = ein("WxT", [DI, R + 2 * DS], BF16)
    WdtT = ein("WdtT", [R, DH], BF16)
    bdt = ein("bdt", [128, NDT])
    Afc = ein("Afc", [128, NDT * DS])
    Abc = ein("Abc", [128, NDT * DS])
    Ddf = ein("Ddf", [DH, 128], BF16)
    Ddb = ein("Ddb", [DH, 128], BF16)
    I128 = ein("I128", [128, 128], BF16)
    G2T = ein("G2T", [2 * DI, 2 * DH], BF16)
    bgate2 = ein("bgate2", [1, 2 * DH], BF16)
    WoT = ein("WoT", [2 * DH, DM], BF16)
    outp = nc.dram_tensor("outp", [DM, L], F32, kind="ExternalOutput")

    bc_dram = nc.dram_tensor("bc_bounce", [2 * DS, L], BF16, kind="Internal")
    g_dram = nc.dram_tensor("g_dram", [1, 2 * DH], F32, kind="Internal")
    cc_zin = nc.dram_tensor("cc_zin", [1, 4], F32, kind="Internal")
    cc_zout = nc.dram_tensor("cc_zout", [1, 8], F32, kind="Internal")
    u_cc_in = nc.dram_tensor("u_cc_in", [1, 2 * DH], F32, kind="Internal")
    u_cc_out = nc.dram_tensor("u_cc_out", [1, 2 * DI], F32, kind="Internal")

    with ExitStack() as ctx:
        tc = ctx.enter_context(tile.TileContext(nc))
        wpool = ctx.enter_context(tc.tile_pool(name="weights", bufs=1))
        apool = ctx.enter_context(tc.tile_pool(name="acts", bufs=1))
        bc_pool = ctx.enter_context(tc.tile_pool(name="bcp", bufs=2))
        da_pool = ctx.enter_context(tc.tile_pool(name="dap", bufs=2))
        dbu_pool = ctx.enter_context(tc.tile_pool(name="dbp", bufs=2))
        h_pool = ctx.enter_context(tc.tile_pool(name="hp", bufs=1))

        def load(name, dram, p, f, eng=None):
            ts = []
            for i in range(0, p, 128):
                pp = min(128, p - i)
                t = wpool.tile([pp, f], dram.dtype, tag=f"{name}{i}", name=f"{name}{i}")
                (eng or nc.sync).dma_start(out=t, in_=dram[i : i + pp, :])
                ts.append(t)
            return ts

        # ---- input loads: first-needed tiles spread over the queues ----
        hs0 = wpool.tile([128, L], BF16, tag="hs0", name="hs0")
        nc.sync.dma_start(out=hs0, in_=hsT[0:128, :])
        hs1_t = wpool.tile([128, L], BF16, tag="hs1", name="hs1")
        nc.scalar.dma_start(out=hs1_t, in_=hsT[128:256, :])
        hs = [hs0, hs1_t]
        winx0 = wpool.tile([128, DI], BF16, tag="winx0", name="winx0")
        nc.gpsimd.dma_start(out=winx0, in_=WinxT[0:128, :])
        winx1 = wpool.tile([128, DI], BF16, tag="winx1", name="winx1")
        nc.sync.dma_start(out=winx1, in_=WinxT[128:256, :])
        winx = [winx0, winx1]
        wx = load("wx", WxT, DI, R + 2 * DS, eng=nc.gpsimd)
        winz = load("winz", WinzT, DM, DH, eng=nc.gpsimd)
        wdt = load("wdt", WdtT, R, DH, eng=nc.gpsimd)
        bdt_s = load("bdt", bdt, 128, NDT, eng=nc.gpsimd)[0]
        af_s = load("afc", Afc, 128, NDT * DS, eng=nc.gpsimd)[0]
        ab_s = load("abc", Abc, 128, NDT * DS, eng=nc.gpsimd)[0]
        ddf = load("ddf", Ddf, DH, 128, eng=nc.gpsimd)
        ddb = load("ddb", Ddb, DH, 128, eng=nc.gpsimd)
        ident = load("ident", I128, 128, 128, eng=nc.gpsimd)[0]
        wo = load("wo", WoT, 2 * DH, DM, eng=nc.gpsimd)
        g2 = load("g2", G2T, 2 * DI, 2 * DH, eng=nc.gpsimd)
        bgate_r = load("bgate2", bgate2, 1, 2 * DH, eng=nc.gpsimd)[0]

        # ---- start barrier (warms the cc path) ----
        zin = apool.tile([1, 4], F32, tag="zin", name="zin")
        nc.vector.memset(zin, 0.0)
        nc.sync.dma_start(out=cc_zin[:, :], in_=zin)
        nc.gpsimd.collective_compute(
            "AllGather", OP.bypass,
            replica_groups=[[0, 1], [2, 3], [4, 5], [6, 7]],
            ins=[cc_zin[:, :]], outs=[cc_zout[:, :]],
        )
        ones11 = apool.tile([1, 1], BF16, tag="ones11", name="ones11")
        nc.vector.memset(ones11, 1.0)

        # prefetch the Silu table before the first in-proj tile lands
        tdum = apool.tile([1, 4], F32, tag="tdum", name="tdum")
        nc.vector.memset(tdum, 0.0)
        nc.scalar.activation(tdum, tdum, AF.Silu)

        # ---- startup: x tiles -> x_proj -> z tiles -> dt (+ group-0 prep) ----
        xT = [apool.tile([128, L], BF16, tag=f"xT{i}", name=f"xT{i}") for i in range(4)]
        zg = [apool.tile([128, L], BF16, tag=f"zg{i}", name=f"zg{i}") for i in range(NDT)]
        dtT = [apool.tile([128, L], BF16, tag=f"dtT{i}", name=f"dtT{i}") for i in range(NDT)]
        sp = [apool.tile([128, L], F32, tag=f"sp{i}", name=f"sp{i}") for i in range(NDT)]
        w2 = [apool.tile([128, L], BF16, tag=f"w2{i}", name=f"w2{i}") for i in range(NDT)]
        g0 = {}
        with tc.tile_pool(name="ps_early", bufs=2, space="PSUM") as ps_early:
            for pc in range(4):
                ps = ps_early.tile([128, L], F32, tag="xz", name="xz")
                for f0, fl in FCH:
                    for kc in range(2):
                        nc.tensor.matmul(
                            ps[:, f0 : f0 + fl],
                            winx[kc][:, pc * 128 : (pc + 1) * 128],
                            hs[kc][:, f0 : f0 + fl],
                            start=(kc == 0), stop=(kc == 1),
                        )
                nc.scalar.activation(xT[pc], ps, AF.Silu)

            # x_proj -> x_dbl [48, L]; bounce B/C rows to DRAM (scalar queue
            # so the sync queue is free to start the broadcast reads)
            xdbl = apool.tile([R + 2 * DS, L], BF16, tag="xdbl", name="xdbl")
            ps = ps_early.tile([R + 2 * DS, L], F32, tag="aux", name="aux")
            for f0, fl in FCH:
                for kc in range(4):
                    nc.tensor.matmul(
                        ps[:, f0 : f0 + fl], wx[kc], xT[kc][:, f0 : f0 + fl],
                        start=(kc == 0), stop=(kc == 3),
                    )
            nc.scalar.activation(xdbl, ps[0 : R + 2 * DS, :], AF.Copy)
            nc.scalar.dma_start(out=bc_dram[:, :], in_=xdbl[R : R + 2 * DS, :])

            # group-0 B/C rows: per-segment broadcasts so each dbu multiply
            # and scan waits only for its own slice
            bcB0 = bc_pool.tile([128, G * L], BF16, tag="bcB", name="bcB")
            bcC0 = bc_pool.tile([128, G * L], BF16, tag="bcC", name="bcC")
            for k in range(G):
                nc.sync.dma_start(
                    out=bcB0[:, k * L : (k + 1) * L],
                    in_=bass.AP(tensor=bc_dram, offset=k * L,
                                ap=[[0, 128], [1, L]]),
                )
            for k in range(G):
                nc.sync.dma_start(
                    out=bcC0[:, k * L : (k + 1) * L],
                    in_=bass.AP(tensor=bc_dram, offset=(DS + k) * L,
                                ap=[[0, 128], [1, L]]),
                )
            g0["bc"] = (bcB0, bcC0)

            # z tiles: Silu still resident in the active table
            for zc in range(NDT):
                ps = ps_early.tile([128, L], F32, tag="xz", name="xz")
                for f0, fl in FCH:
                    for kc in range(2):
                        nc.tensor.matmul(
                            ps[:, f0 : f0 + fl],
                            winz[kc][:, zc * 128 : (zc + 1) * 128],
                            hs[kc][:, f0 : f0 + fl],
                            start=(kc == 0), stop=(kc == 1),
                        )
                nc.scalar.activation(zg[zc], ps, AF.Silu)

            # dt = ln(exp(q + b_dt) + 1); per dtile, then its group-0 dA
            # exps + w2 + dbu so the first scan's inputs form ASAP
            dtps = []
            for dtc in range(NDT):
                ps = ps_early.tile([128, L], F32, tag="aux", name="aux")
                for f0, fl in FCH:
                    nc.tensor.matmul(
                        ps[:, f0 : f0 + fl],
                        wdt[0][:, dtc * 128 : (dtc + 1) * 128],
                        xdbl[0:R, f0 : f0 + fl], start=True, stop=True,
                    )
                dtps.append(ps)
            # group-0 dA exps, dbu multiplies and scans, interleaved
            # per-segment so the DVE's in-order queue never stalls behind a
            # segment whose broadcast slice hasn't landed yet
            g0["h"] = {}
            for dtc in range(NDT):
                nc.scalar.activation(
                    sp[dtc], dtps[dtc], AF.Exp, bias=bdt_s[:, dtc : dtc + 1]
                )
                nc.scalar.activation(dtT[dtc], sp[dtc], AF.Ln, bias=1.0)
                daf = da_pool.tile([128, GW], F32, tag=f"daf{dtc}", name=f"daf{dtc}")
                for k in range(G):
                    col = dtc * DS + k
                    nc.scalar.activation(
                        _seg(daf, k), dtT[dtc], AF.Exp,
                        scale=af_s[:, col : col + 1],
                    )
                if shared_a:
                    dab = daf
                else:
                    dab = da_pool.tile([128, GW], F32, tag=f"dab{dtc}", name=f"dab{dtc}")
                    for k in range(G):
                        col = dtc * DS + k
                        nc.scalar.activation(
                            _seg(dab, k), dtT[dtc], AF.Exp,
                            scale=ab_s[:, col : col + 1],
                        )
                nc.vector.tensor_mul(w2[dtc], dtT[dtc], xT[dtc])
                dbu = dbu_pool.tile([128, GW], BF16, tag=f"dbu{dtc}", name=f"dbu{dtc}")
                hf = h_pool.tile([128, GW], BF16, tag=f"h0{dtc}", name=f"h0{dtc}")
                hb = h_pool.tile([128, GW], BF16, tag=f"h1{dtc}", name=f"h1{dtc}")
                g0["h"][(0, dtc)] = hf
                g0["h"][(1, dtc)] = hb
                for k in range(G):
                    nc.vector.tensor_mul(_seg(dbu, k), w2[dtc],
                                         bcB0[:, k * L : (k + 1) * L])
                    nc.vector.tensor_tensor_scan(
                        _seg(hf, k), _seg(daf, k), _seg(dbu, k),
                        0.0, OP.mult, OP.add,
                    )
                    nc.vector.tensor_tensor_scan(
                        _seg(hb, k)[:, ::-1], _seg(dab, k)[:, ::-1],
                        _seg(dbu, k)[:, ::-1], 0.0, OP.mult, OP.add,
                    )

        # ---- grouped scan loop ----
        ypsum = {}
        m_sb = apool.tile([128, 4], F32, tag="m", name="m")  # cols f0,f1,b0,b1
        yg = {}
        with tc.tile_pool(name="ps_y", bufs=1, space="PSUM") as ps_y:
            for dr in range(2):
                for dtc in range(NDT):
                    yp = ps_y.tile([128, L], F32, tag=f"y{dr}{dtc}", name=f"y{dr}{dtc}")
                    ypsum[(dr, dtc)] = yp
                    dd = (ddf if dr == 0 else ddb)[dtc]
                    for f0, fl in FCH:
                        nc.tensor.matmul(
                            yp[:, f0 : f0 + fl], dd, xT[dtc][:, f0 : f0 + fl],
                            start=True, stop=False, skip_group_check=True,
                        )

            # 3-dim AP over the G segments of a grouped tile (skips pads)
            def m3(t, stride):
                return bass.AP(tensor=t.tensor, offset=t.offset,
                               ap=[t.ap[0], [stride, G], [1, L]])

            # zero the inter-segment pad columns so a single merged scan
            # resets its state (a=0, b=0) crossing each boundary — in both
            # directions; done on first use of each physical pool buffer
            def zero_pads(t):
                for k in range(G - 1):
                    nc.vector.memset(t[:, k * SEG + L : (k + 1) * SEG], 0.0)

            MW = (G - 1) * SEG + L  # merged scan width (ends at seg G-1)

            for g in range(NG):
                n0 = g * G
                if g == 0:
                    bcB, bcC = g0["bc"]
                else:
                    bcB = bc_pool.tile([128, G * L], BF16, tag="bcB", name="bcB")
                    bcC = bc_pool.tile([128, G * L], BF16, tag="bcC", name="bcC")
                    nc.sync.dma_start(
                        out=bcB,
                        in_=bass.AP(tensor=bc_dram, offset=n0 * L,
                                    ap=[[0, 128], [L, G], [1, L]]),
                    )
                    nc.sync.dma_start(
                        out=bcC,
                        in_=bass.AP(tensor=bc_dram, offset=(DS + n0) * L,
                                    ap=[[0, 128], [L, G], [1, L]]),
                    )

                for dtc in range(NDT):
                    if g == 0:
                        daf = dab = dbu = None
                    else:
                        daf = da_pool.tile([128, GW], F32, tag=f"daf{dtc}", name=f"daf{dtc}")
                        if g in (1, 2):
                            zero_pads(daf)
                        for k in range(G):
                            col = dtc * DS + n0 + k
                            nc.scalar.activation(
                                _seg(daf, k), dtT[dtc], AF.Exp,
                                scale=af_s[:, col : col + 1],
                            )
                        if shared_a:
                            dab = daf
                        else:
                            dab = da_pool.tile([128, GW], F32, tag=f"dab{dtc}", name=f"dab{dtc}")
                            if g in (1, 2):
                                zero_pads(dab)
                            for k in range(G):
                                col = dtc * DS + n0 + k
                                nc.scalar.activation(
                                    _seg(dab, k), dtT[dtc], AF.Exp,
                                    scale=ab_s[:, col : col + 1],
                                )
                        dbu = dbu_pool.tile([128, GW], BF16, tag=f"dbu{dtc}", name=f"dbu{dtc}")
                        if g in (1, 2):
                            zero_pads(dbu)
                        # one multiply for all G segments (w2 repeated via a
                        # stride-0 dim); ~360ns fixed TT overhead amortized
                        nc.vector.tensor_tensor(
                            m3(dbu, SEG), m3(w2[dtc], 0), m3(bcB, L), OP.mult
                        )

                    for dr in range(2):
                        if g == 0:
                            # scans already emitted in the startup section
                            h = g0["h"][(dr, dtc)]
                        else:
                            h = h_pool.tile([128, GW], BF16, tag=f"h{dr}{dtc}", name=f"h{dr}{dtc}")
                            # one merged scan across all G segments; the
                            # zeroed pads reset the state at each boundary
                            if dr == 0:
                                nc.vector.tensor_tensor_scan(
                                    h[:, 0:MW], daf[:, 0:MW], dbu[:, 0:MW],
                                    0.0, OP.mult, OP.add,
                                )
                            else:
                                nc.vector.tensor_tensor_scan(
                                    h[:, 0:MW][:, ::-1], dab[:, 0:MW][:, ::-1],
                                    dbu[:, 0:MW][:, ::-1], 0.0, OP.mult, OP.add,
                                )
                        # merged h *= C over all segments
                        nc.vector.tensor_tensor(
                            m3(h, SEG), m3(h, SEG), m3(bcC, L), OP.mult
                        )
                        yp = ypsum[(dr, dtc)]
                        for k in range(G):
                            for f0, fl in FCH:
                                nc.tensor.matmul(
                                    yp[:, f0 : f0 + fl],
                                    ident,
                                    _seg(h, k, f0, fl),
                                    start=False,
                                    stop=(g == NG - 1 and k == G - 1),
                                    skip_group_check=True,
                                )
                        # last group: yg = y*zg with the pooled sum riding
                        # the same DVE op's accumulator, then that column of
                        # the exchange payload goes out immediately — only
                        # the last 512B column sits on the collective's
                        # critical path (and the partner core speeds up
                        # symmetrically, so the AllGather starts earlier)
                        if g == NG - 1:
                            c = 2 * dr + dtc
                            t = apool.tile([128, L], BF16, tag=f"yg{dr}{dtc}",
                                           name=f"yg{dr}{dtc}")
                            yg[(dr, dtc)] = t
                            nc.vector.scalar_tensor_tensor(
                                t, ypsum[(dr, dtc)], 1.0, zg[dtc],
                                OP.mult, OP.mult,
                                accum_out=m_sb[:, c : c + 1],
                            )
                            nc.gpsimd.dma_start(
                                out=bass.AP(tensor=u_cc_in, offset=128 * c,
                                            ap=[[1, 128]]),
                                in_=m_sb[:, c : c + 1],
                            )

        # ---- exchange pooled vector (fp32): u2[p, j] = v_full[p + 128j] ----
        with tc.tile_pool(name="ps_tail", bufs=1, space="PSUM") as ps_tail:
            nc.gpsimd.collective_compute(
                "AllGather", OP.bypass,
                replica_groups=[[0, 1], [2, 3], [4, 5], [6, 7]],
                ins=[u_cc_in[:, :]], outs=[u_cc_out[:, :]],
            )
            # prefetch the Sigmoid table while the collective runs; reading
            # m_sb pins the op after the accumulates (the scheduler would
            # otherwise float a dependency-free op anywhere)
            nc.scalar.activation(tdum, m_sb[0:1, 0:4], AF.Sigmoid)
            # fetch the gathered vector in two halves so the gate matvec's
            # first four columns start ~1us before the second half lands
            u2 = apool.tile([128, 8], F32, tag="u2", name="u2")
            u2b = apool.tile([128, 8], BF16, tag="u2b", name="u2b")
            for hh in range(2):
                nc.sync.dma_start(
                    out=u2[:, 4 * hh : 4 * hh + 4],
                    in_=bass.AP(tensor=u_cc_out, offset=512 * hh,
                                ap=[[1, 128], [128, 4]]),
                )
                nc.scalar.activation(
                    u2b[:, 4 * hh : 4 * hh + 4], u2[:, 4 * hh : 4 * hh + 4],
                    AF.Copy,
                )

            # ---- gate: q = G2^T v + b (bias via ones-row matmul); sigmoid ----
            vps = ps_tail.tile([1, 2 * DH], F32, tag="vps", name="vps")
            nc.tensor.matmul(vps, ones11, bgate_r, start=True, stop=False)
            for kc in range(8):
                nc.tensor.matmul(
                    vps, u2b[:, kc : kc + 1], g2[kc], start=False, stop=(kc == 7)
                )
            g_row = apool.tile([1, 2 * DH], F32, tag="grow", name="grow")
            nc.scalar.activation(g_row, vps, AF.Sigmoid)
            # reshape [1,512] -> [128,4] per-partition scalars: four
            # [1,128]->[128,1] SBUF-to-SBUF column DMAs (3-dim APs balance;
            # the single 4-dim reshape DMA does not, and a PE-transpose
            # variant intermittently produced NaN / wedged the device)
            g4 = apool.tile([128, 4], F32, tag="g4", name="g4")
            for j in range(4):
                nc.sync.dma_start(
                    out=g4[:, j : j + 1],
                    in_=g_row[0:1, j * 128 : (j + 1) * 128],
                )

            # fold the gate into the out-proj weight rows (channel-parallel)
            wos = [apool.tile([128, DM], BF16, tag=f"wos{i}", name=f"wos{i}")
                   for i in range(4)]
            for kc in range(4):
                nc.vector.tensor_scalar_mul(wos[kc], wo[kc], g4[:, kc : kc + 1])

            out_sb = [apool.tile([128, L], F32, tag=f"o{i}", name=f"o{i}")
                      for i in range(2)]
            for pc in range(2):
                ops_ = ps_tail.tile([128, L], F32, tag="ops", name="ops")
                for f0, fl in FCH:
                    for kc in range(4):
                        nc.tensor.matmul(
                            ops_[:, f0 : f0 + fl],
                            wos[kc][:, pc * 128 : (pc + 1) * 128],
                            yg[(kc // 2, kc % 2)][:, f0 : f0 + fl],
                            start=(kc == 0), stop=(kc == 3),
                        )
                    # copy + store per chunk so the last DMA tail is short
                    nc.scalar.activation(
                        out_sb[pc][:, f0 : f0 + fl], ops_[:, f0 : f0 + fl], AF.Copy
                    )
                    nc.sync.dma_start(
                        out=outp[pc * 128 : (pc + 1) * 128, f0 : f0 + fl],
                        in_=out_sb[pc][:, f0 : f0 + fl],
                    )

    nc.finalize()
    return nc


_NC_CACHE = {}


def _get_module(shared_a: bool):
    if shared_a not in _NC_CACHE:
        _NC_CACHE[shared_a] = _build_module(shared_a)
    return _NC_CACHE[shared_a]


def _diag_stack(d):
    out = np.zeros((DH, 128), dtype=np.float32)
    for t in range(NDT):
        out[t * 128 : (t + 1) * 128, :] = np.diag(d[t * 128 : (t + 1) * 128])
    return out


def kernel(**inputs):
    inp = {k: np.asarray(v, dtype=np.float32) for k, v in inputs.items()}
    hs = inp["hidden_states"]
    W_in, W_x, W_dt = inp["W_in"], inp["W_xproj"], inp["W_dt"]
    b_dt = inp["b_dt"]
    A_f = -np.exp(inp["A_log_f"])
    A_b = -np.exp(inp["A_log_b"])
    D_f, D_b = inp["D_f"], inp["D_b"]
    W_g, b_g = inp["W_global"], inp["b_global"]
    W_gate, b_gate = inp["W_gate"], inp["b_gate"]
    W_out = inp["W_out"]

    shared_a = bool(np.array_equal(A_f, A_b))
    I = np.eye(128, dtype=np.float32)
    bf = ml_dtypes.bfloat16
    in_maps = []
    for core in range(8):
        b, h = core // 2, core % 2
        o = h * DH
        perm = np.r_[o : o + DH, (DH - o) % DI : (DH - o) % DI + DH]
        ownc = np.r_[o : o + DH, DI + o : DI + o + DH]
        ccorder = np.r_[0:DH, DI : DI + DH, DH:DI, DI + DH : 2 * DI]

        def acol(A):
            a = A[o : o + DH].reshape(NDT, 128, DS)
            return np.ascontiguousarray(a.transpose(1, 0, 2).reshape(128, NDT * DS))

        m = {
            "hsT": np.ascontiguousarray(hs[b].T).astype(bf),
            "WinxT": np.ascontiguousarray(W_in[:DI][perm].T).astype(bf),
            "WinzT": np.ascontiguousarray(W_in[DI + o : DI + o + DH].T).astype(bf),
            "WxT": np.ascontiguousarray(W_x[:, perm].T).astype(bf),
            "WdtT": np.ascontiguousarray(W_dt[o : o + DH].T).astype(bf),
            "bdt": np.ascontiguousarray(b_dt[o : o + DH].reshape(NDT, 128).T),
            "Afc": acol(A_f),
            "Abc": acol(A_b),
            "Ddf": _diag_stack(D_f[o : o + DH]).astype(bf),
            "Ddb": _diag_stack(D_b[o : o + DH]).astype(bf),
            "I128": I.astype(bf),
            "G2T": np.ascontiguousarray(
                (W_gate[ownc] @ W_g[:, ccorder] / np.float32(L)).T
            ).astype(bf),
            "bgate2": np.ascontiguousarray(
                (b_gate[ownc] + W_gate[ownc] @ b_g).reshape(1, 512)
            ).astype(bf),
            "WoT": np.ascontiguousarray(W_out[:, ownc].T).astype(bf),
        }
        in_maps.append(m)

    nc = _get_module(shared_a)
    res = run_bass_kernel_spmd(nc, in_maps, core_ids=list(range(8)))
    outs = res.results
    out = np.zeros((B, L, DM), dtype=np.float32)
    for b in range(B):
        part = outs[2 * b]["outp"] + outs[2 * b + 1]["outp"]
        out[b] = part.T
    return out


# revision 39
# speedup vs baseline: 1.0109x; 1.0109x over previous
"""Bidirectional DSS/Mamba block on 8 trn2 cores (Bass/Tile) — v8.

Sharding: core = (batch b = core//2, d_inner half = core%2).

HW-measured basis: the DVE scan runs ~2.25ns/elem regardless of dtype
(recurrence feedback latency), so the 64 [128,900]-equivalent scans are a
~125us floor; everything else is arranged to saturate the DVE from ~30us
to the end of the scan phase and to shrink the exchange + tail latency.
Measured facts driving this version (vs the 266us v2 baseline):
- DVE tensor_tensor has a ~140-360ns fixed overhead: groups 1-3 run ONE
  merged scan per (dir, dtile) across all 4 segments (the zeroed 4-column
  pads between SEG=904 segments reset the state in both directions), one
  merged h*=C multiply, and one merged dbu multiply (w2 repeated via a
  stride-0 AP dim). Validated bit-exact against numpy on hardware.
- One activation table is active at a time (1.28us per switch); blanking
  the exp-only/ln-only sets steers Exp AND Ln onto
  natural_log_exp_and_others so the dt chain exp->ln->dA-exps runs with a
  single load (verified: no reload between Ln and the dA exps).
- dt = ln(exp(q + b_dt) + 1): the +1 rides the Ln bias port.
- The [128, 4*900] broadcast DMA takes ~9us to land (921KB); group 0's
  B and C rows are fetched as per-segment DMAs and its dbu multiplies and
  scans are hoisted into the startup section interleaved per segment
  [dbu_k, scan_fwd_k, scan_bwd_k] — otherwise the DVE's in-order queue
  head-of-line blocks on broadcast slices that haven't landed.
- z in-proj sits between x_proj and the dt matmuls: its Silu runs under
  the still-loaded initial Silu table (a mid-scan-loop placement caused
  extra table loads; the tile scheduler reorders dependency-free code).
- The yg multiply + pooled-sum accumulate run inside the scan loop as
  each (dir, dtile) PSUM accumulator completes.
- The pooled exchange is fp32 (collective time is latency-noise, not
  bytes); the gate matvec accumulates its bias via a ones-row matmul, the
  sigmoid runs on ACT with its table prefetch pinned to m_sb (a
  dependency-free op would be scheduler-hoisted into the scan phase), and
  the gate scales the W_out rows ([128,256] per-partition muls) instead
  of the yg tiles.
- No DVE-paced "warm" chain: the tile scheduler hoists dependency-free
  work, so such a chain steals mid-scan DVE cycles (measured +9us).
"""

import sys

sys.path.insert(0, "/opt/trn_rl_repo")

from contextlib import ExitStack

import ml_dtypes
import numpy as np

import concourse.bass as bass
import concourse.bacc as bacc
import concourse.tile as tile
from concourse import mybir
from concourse.bass_utils import run_bass_kernel_spmd
from concourse.hw_specs import get_activation_tables

F32 = mybir.dt.float32
BF16 = mybir.dt.bfloat16
AF = mybir.ActivationFunctionType
OP = mybir.AluOpType

B, L, DM, DS, DI, R = 4, 900, 256, 16, 512, 16
DH = DI // 2          # d_inner channels per core
NDT = DH // 128       # 128-channel tiles per core (2)
FCH = [(0, 512), (512, L - 512)]  # PSUM-bank-aligned L chunks
SEG = 904             # aligned segment stride in grouped tiles (16B)


class _Bacc(bacc.Bacc):
    """Bacc that steers Exp and Ln onto the combined exp+ln table set.

    First-fit selection would pick exp_and_others / natural_log and
    thrash the single active table across the dt chain. Blanking those
    sets (positions preserved, so the emitted act_func_set_id still
    indexes act_info.json) makes both functions resolve to
    natural_log_exp_and_others.
    """

    def insert_act_table_loads(self):
        has_activation = any(
            isinstance(i, mybir.InstActivation)
            for b in self.main_func.blocks
            for i in b.instructions
        )
        if not has_activation:
            return
        drop = {"exp_and_others", "natural_log"}
        tables = [
            (n, (set() if n in drop else s))
            for n, s in get_activation_tables(self.m.arch).items()
        ]
        bacc._bass_rust.insert_act_table_loads(self, tables)


def _seg(t, k, f0=0, fl=L):
    """Slice segment k's [f0:f0+fl] columns of a grouped [128, G*SEG] tile."""
    return t[:, k * SEG + f0 : k * SEG + f0 + fl]


def _build_module(shared_a: bool):
    G = 4 if shared_a else 2      # n-group size (SBUF bound when A_f != A_b)
    NG = DS // G
    GW = G * SEG                  # grouped tile width

    nc = _Bacc("TRN2", num_devices=8)

    ein = lambda n, s, d=F32: nc.dram_tensor(n, s, d, kind="ExternalInput")
    hsT = ein("hsT", [DM, L], BF16)
    WinxT = ein("WinxT", [DM, DI], BF16)
    WinzT = ein("WinzT", [DM, DH], BF16)
    WxT = ein("WxT", [DI, R + 2 * DS], BF16)
    WdtT = ein("WdtT", [R, DH], BF16)
    bdt = ein("bdt", [128, NDT])
    Afc = ein("Afc", [128, NDT * DS])
    Abc = ein("Abc", [128, NDT * DS])
    Ddf = ein("Ddf", [DH, 128], BF16)
    Ddb = ein("Ddb", [DH, 128], BF16)
    I128 = ein("I128", [128, 128], BF16)
    G2T = ein("G2T", [2 * DI, 2 * DH], BF16)
    bgate2 = ein("bgate2", [1, 2 * DH], BF16)
    WoT = ein("WoT", [2 * DH, DM], BF16)
    outp = nc.dram_tensor("outp", [DM, L], F32, kind="ExternalOutput")

    bc_dram = nc.dram_tensor("bc_bounce", [2 * DS, L], BF16, kind="Internal")
    g_dram = nc.dram_tensor("g_dram", [1, 2 * DH], F32, kind="Internal")
    cc_zin = nc.dram_tensor("cc_zin", [1, 4], F32, kind="Internal")
    cc_zout = nc.dram_tensor("cc_zout", [1, 8], F32, kind="Internal")
    u_cc_in = nc.dram_tensor("u_cc_in", [1, 2 * DH], F32, kind="Internal")
    u_cc_out = nc.dram_tensor("u_cc_out", [1, 2 * DI], F32, kind="Internal")

    with ExitStack() as ctx:
        tc = ctx.enter_context(tile.TileContext(nc))
        wpool = ctx.enter_context(tc.tile_pool(name="weights", bufs=1))
        apool = ctx.enter_context(tc.tile_pool(name="acts", bufs=1))
        bc_pool = ctx.enter_context(tc.tile_pool(name="bcp", bufs=2))
        da_pool = ctx.enter_context(tc.tile_pool(name="dap", bufs=2))
        dbu_pool = ctx.enter_context(tc.tile_pool(name="dbp", bufs=2))
        h_pool = ctx.enter_context(tc.tile_pool(name="hp", bufs=1))

        def load(name, dram, p, f, eng=None):
            ts = []
            for i in range(0, p, 128):
                pp = min(128, p - i)
                t = wpool.tile([pp, f], dram.dtype, tag=f"{name}{i}", name=f"{name}{i}")
                (eng or nc.sync).dma_start(out=t, in_=dram[i : i + pp, :])
                ts.append(t)
            return ts

        # ---- input loads: first-needed tiles spread over the queues ----
        hs0 = wpool.tile([128, L], BF16, tag="hs0", name="hs0")
        nc.sync.dma_start(out=hs0, in_=hsT[0:128, :])
        hs1_t = wpool.tile([128, L], BF16, tag="hs1", name="hs1")
        nc.scalar.dma_start(out=hs1_t, in_=hsT[128:256, :])
        hs = [hs0, hs1_t]
        winx0 = wpool.tile([128, DI], BF16, tag="winx0", name="winx0")
        nc.gpsimd.dma_start(out=winx0, in_=WinxT[0:128, :])
        winx1 = wpool.tile([128, DI], BF16, tag="winx1", name="winx1")
        nc.sync.dma_start(out=winx1, in_=WinxT[128:256, :])
        winx = [winx0, winx1]
        wx = load("wx", WxT, DI, R + 2 * DS, eng=nc.gpsimd)
        winz = load("winz", WinzT, DM, DH, eng=nc.gpsimd)
        wdt = load("wdt", WdtT, R, DH, eng=nc.gpsimd)
        bdt_s = load("bdt", bdt, 128, NDT, eng=nc.gpsimd)[0]
        af_s = load("afc", Afc, 128, NDT * DS, eng=nc.gpsimd)[0]
        ab_s = load("abc", Abc, 128, NDT * DS, eng=nc.gpsimd)[0]
        ddf = load("ddf", Ddf, DH, 128, eng=nc.gpsimd)
        ddb = load("ddb", Ddb, DH, 128, eng=nc.gpsimd)
        ident = load("ident", I128, 128, 128, eng=nc.gpsimd)[0]
        wo = load("wo", WoT, 2 * DH, DM, eng=nc.gpsimd)
        g2 = load("g2", G2T, 2 * DI, 2 * DH, eng=nc.gpsimd)
        bgate_r = load("bgate2", bgate2, 1, 2 * DH, eng=nc.gpsimd)[0]

        # ---- start barrier (warms the cc path) ----
        zin = apool.tile([1, 4], F32, tag="zin", name="zin")
        nc.vector.memset(zin, 0.0)
        nc.sync.dma_start(out=cc_zin[:, :], in_=zin)
        nc.gpsimd.collective_compute(
            "AllGather", OP.bypass,
            replica_groups=[[0, 1], [2, 3], [4, 5], [6, 7]],
            ins=[cc_zin[:, :]], outs=[cc_zout[:, :]],
        )
        ones11 = apool.tile([1, 1], BF16, tag="ones11", name="ones11")
        nc.vector.memset(ones11, 1.0)

        # prefetch the Silu table before the first in-proj tile lands
        tdum = apool.tile([1, 4], F32, tag="tdum", name="tdum")
        nc.vector.memset(tdum, 0.0)
        nc.scalar.activation(tdum, tdum, AF.Silu)

        # ---- startup: x tiles -> x_proj -> z tiles -> dt (+ group-0 prep) ----
        xT = [apool.tile([128, L], BF16, tag=f"xT{i}", name=f"xT{i}") for i in range(4)]
        zg = [apool.tile([128, L], BF16, tag=f"zg{i}", name=f"zg{i}") for i in range(NDT)]
        dtT = [apool.tile([128, L], BF16, tag=f"dtT{i}", name=f"dtT{i}") for i in range(NDT)]
        sp = [apool.tile([128, L], F32, tag=f"sp{i}", name=f"sp{i}") for i in range(NDT)]
        w2 = [apool.tile([128, L], BF16, tag=f"w2{i}", name=f"w2{i}") for i in range(NDT)]
        g0 = {}
        with tc.tile_pool(name="ps_early", bufs=2, space="PSUM") as ps_early:
            for pc in range(4):
                ps = ps_early.tile([128, L], F32, tag="xz", name="xz")
                for f0, fl in FCH:
                    for kc in range(2):
                        nc.tensor.matmul(
                            ps[:, f0 : f0 + fl],
                            winx[kc][:, pc * 128 : (pc + 1) * 128],
                            hs[kc][:, f0 : f0 + fl],
                            start=(kc == 0), stop=(kc == 1),
                        )
                nc.scalar.activation(xT[pc], ps, AF.Silu)

            # x_proj -> x_dbl [48, L]; bounce B/C rows to DRAM (scalar queue
            # so the sync queue is free to start the broadcast reads)
            xdbl = apool.tile([R + 2 * DS, L], BF16, tag="xdbl", name="xdbl")
            ps = ps_early.tile([R + 2 * DS, L], F32, tag="aux", name="aux")
            for f0, fl in FCH:
                for kc in range(4):
                    nc.tensor.matmul(
                        ps[:, f0 : f0 + fl], wx[kc], xT[kc][:, f0 : f0 + fl],
                        start=(kc == 0), stop=(kc == 3),
                    )
            nc.scalar.activation(xdbl, ps[0 : R + 2 * DS, :], AF.Copy)
            nc.scalar.dma_start(out=bc_dram[:, :], in_=xdbl[R : R + 2 * DS, :])

            # group-0 B/C rows: per-segment broadcasts so each dbu multiply
            # and scan waits only for its own slice
            bcB0 = bc_pool.tile([128, G * L], BF16, tag="bcB", name="bcB")
            bcC0 = bc_pool.tile([128, G * L], BF16, tag="bcC", name="bcC")
            for k in range(G):
                nc.sync.dma_start(
                    out=bcB0[:, k * L : (k + 1) * L],
                    in_=bass.AP(tensor=bc_dram, offset=k * L,
                                ap=[[0, 128], [1, L]]),
                )
            for k in range(G):
                nc.sync.dma_start(
                    out=bcC0[:, k * L : (k + 1) * L],
                    in_=bass.AP(tensor=bc_dram, offset=(DS + k) * L,
                                ap=[[0, 128], [1, L]]),
                )
            g0["bc"] = (bcB0, bcC0)

            # z tiles: Silu still resident in the active table
            for zc in range(NDT):
                ps = ps_early.tile([128, L], F32, tag="xz", name="xz")
                for f0, fl in FCH:
                    for kc in range(2):
                        nc.tensor.matmul(
                            ps[:, f0 : f0 + fl],
                            winz[kc][:, zc * 128 : (zc + 1) * 128],
                            hs[kc][:, f0 : f0 + fl],
                            start=(kc == 0), stop=(kc == 1),
                        )
                nc.scalar.activation(zg[zc], ps, AF.Silu)

            # dt = ln(exp(q + b_dt) + 1); per dtile, then its group-0 dA
            # exps + w2 + dbu so the first scan's inputs form ASAP
            dtps = []
            for dtc in range(NDT):
                ps = ps_early.tile([128, L], F32, tag="aux", name="aux")
                for f0, fl in FCH:
                    nc.tensor.matmul(
                        ps[:, f0 : f0 + fl],
                        wdt[0][:, dtc * 128 : (dtc + 1) * 128],
                        xdbl[0:R, f0 : f0 + fl], start=True, stop=True,
                    )
                dtps.append(ps)
            # group-0 dA exps, dbu multiplies and scans, interleaved
            # per-segment so the DVE's in-order queue never stalls behind a
            # segment whose broadcast slice hasn't landed yet
            g0["h"] = {}
            for dtc in range(NDT):
                nc.scalar.activation(
                    sp[dtc], dtps[dtc], AF.Exp, bias=bdt_s[:, dtc : dtc + 1]
                )
                nc.scalar.activation(dtT[dtc], sp[dtc], AF.Ln, bias=1.0)
                daf = da_pool.tile([128, GW], F32, tag=f"daf{dtc}", name=f"daf{dtc}")
                for k in range(G):
                    col = dtc * DS + k
                    nc.scalar.activation(
                        _seg(daf, k), dtT[dtc], AF.Exp,
                        scale=af_s[:, col : col + 1],
                    )
                if shared_a:
                    dab = daf
                else:
                    dab = da_pool.tile([128, GW], F32, tag=f"dab{dtc}", name=f"dab{dtc}")
                    for k in range(G):
                        col = dtc * DS + k
                        nc.scalar.activation(
                            _seg(dab, k), dtT[dtc], AF.Exp,
                            scale=ab_s[:, col : col + 1],
                        )
                nc.vector.tensor_mul(w2[dtc], dtT[dtc], xT[dtc])
                dbu = dbu_pool.tile([128, GW], BF16, tag=f"dbu{dtc}", name=f"dbu{dtc}")
                hf = h_pool.tile([128, GW], BF16, tag=f"h0{dtc}", name=f"h0{dtc}")
                hb = h_pool.tile([128, GW], BF16, tag=f"h1{dtc}", name=f"h1{dtc}")
                g0["h"][(0, dtc)] = hf
                g0["h"][(1, dtc)] = hb
                for k in range(G):
                    nc.vector.tensor_mul(_seg(dbu, k), w2[dtc],
                                         bcB0[:, k * L : (k + 1) * L])
                    nc.vector.tensor_tensor_scan(
                        _seg(hf, k), _seg(daf, k), _seg(dbu, k),
                        0.0, OP.mult, OP.add,
                    )
                    nc.vector.tensor_tensor_scan(
                        _seg(hb, k)[:, ::-1], _seg(dab, k)[:, ::-1],
                        _seg(dbu, k)[:, ::-1], 0.0, OP.mult, OP.add,
                    )

        # ---- grouped scan loop ----
        ypsum = {}
        m_sb = apool.tile([128, 4], F32, tag="m", name="m")  # cols f0,f1,b0,b1
        yg = {}
        with tc.tile_pool(name="ps_y", bufs=1, space="PSUM") as ps_y:
            for dr in range(2):
                for dtc in range(NDT):
                    yp = ps_y.tile([128, L], F32, tag=f"y{dr}{dtc}", name=f"y{dr}{dtc}")
                    ypsum[(dr, dtc)] = yp
                    dd = (ddf if dr == 0 else ddb)[dtc]
                    for f0, fl in FCH:
                        nc.tensor.matmul(
                            yp[:, f0 : f0 + fl], dd, xT[dtc][:, f0 : f0 + fl],
                            start=True, stop=False, skip_group_check=True,
                        )

            # 3-dim AP over the G segments of a grouped tile (skips pads)
            def m3(t, stride):
                return bass.AP(tensor=t.tensor, offset=t.offset,
                               ap=[t.ap[0], [stride, G], [1, L]])

            # zero the inter-segment pad columns so a single merged scan
            # resets its state (a=0, b=0) crossing each boundary — in both
            # directions; done on first use of each physical pool buffer
            def zero_pads(t):
                for k in range(G - 1):
                    nc.vector.memset(t[:, k * SEG + L : (k + 1) * SEG], 0.0)

            MW = (G - 1) * SEG + L  # merged scan width (ends at seg G-1)

            for g in range(NG):
                n0 = g * G
                if g == 0:
                    bcB, bcC = g0["bc"]
                else:
                    bcB = bc_pool.tile([128, G * L], BF16, tag="bcB", name="bcB")
                    bcC = bc_pool.tile([128, G * L], BF16, tag="bcC", name="bcC")
                    nc.sync.dma_start(
                        out=bcB,
                        in_=bass.AP(tensor=bc_dram, offset=n0 * L,
                                    ap=[[0, 128], [L, G], [1, L]]),
                    )
                    nc.sync.dma_start(
                        out=bcC,
                        in_=bass.AP(tensor=bc_dram, offset=(DS + n0) * L,
                                    ap=[[0, 128], [L, G], [1, L]]),
                    )

                for dtc in range(NDT):
                    if g == 0:
                        daf = dab = dbu = None
                    else:
                        daf = da_pool.tile([128, GW], F32, tag=f"daf{dtc}", name=f"daf{dtc}")
                        if g in (1, 2):
                            zero_pads(daf)
                        for k in range(G):
                            col = dtc * DS + n0 + k
                            nc.scalar.activation(
                                _seg(daf, k), dtT[dtc], AF.Exp,
                                scale=af_s[:, col : col + 1],
                            )
                        if shared_a:
                            dab = daf
                        else:
                            dab = da_pool.tile([128, GW], F32, tag=f"dab{dtc}", name=f"dab{dtc}")
                            if g in (1, 2):
                                zero_pads(dab)
                            for k in range(G):
                                col = dtc * DS + n0 + k
                                nc.scalar.activation(
                                    _seg(dab, k), dtT[dtc], AF.Exp,
                                    scale=ab_s[:, col : col + 1],
                                )
                        dbu = dbu_pool.tile([128, GW], BF16, tag=f"dbu{dtc}", name=f"dbu{dtc}")
                        if g in (1, 2):
                            zero_pads(dbu)
                        # one multiply for all G segments (w2 repeated via a
                        # stride-0 dim); ~360ns fixed TT overhead amortized
                        nc.vector.tensor_tensor(
                            m3(dbu, SEG), m3(w2[dtc], 0), m3(bcB, L), OP.mult
                        )

                    for dr in range(2):
                        if g == 0:
                            # scans already emitted in the startup section
                            h = g0["h"][(dr, dtc)]
                        else:
                            h = h_pool.tile([128, GW], BF16, tag=f"h{dr}{dtc}", name=f"h{dr}{dtc}")
                            # one merged scan across all G segments; the
                            # zeroed pads reset the state at each boundary
                            if dr == 0:
                                nc.vector.tensor_tensor_scan(
                                    h[:, 0:MW], daf[:, 0:MW], dbu[:, 0:MW],
                                    0.0, OP.mult, OP.add,
                                )
                            else:
                                nc.vector.tensor_tensor_scan(
                                    h[:, 0:MW][:, ::-1], dab[:, 0:MW][:, ::-1],
                                    dbu[:, 0:MW][:, ::-1], 0.0, OP.mult, OP.add,
                                )
                        # merged h *= C over all segments
                        nc.vector.tensor_tensor(
                            m3(h, SEG), m3(h, SEG), m3(bcC, L), OP.mult
                        )
                        yp = ypsum[(dr, dtc)]
                        for k in range(G):
                            for f0, fl in FCH:
                                nc.tensor.matmul(
                                    yp[:, f0 : f0 + fl],
                                    ident,
                                    _seg(h, k, f0, fl),
                                    start=False,
                                    stop=(g == NG - 1 and k == G - 1),
                                    skip_group_check=True,
                                )
                        # last group: yg = y*zg with the pooled sum riding
                        # the same DVE op's accumulator, then that column of
                        # the exchange payload goes out immediately — only
                        # the last 512B column sits on the collective's
                        # critical path (and the partner core speeds up
                        # symmetrically, so the AllGather starts earlier)
                        if g == NG - 1:
                            c = 2 * dr + dtc
                            t = apool.tile([128, L], BF16, tag=f"yg{dr}{dtc}",
                                           name=f"yg{dr}{dtc}")
                            yg[(dr, dtc)] = t
                            nc.vector.scalar_tensor_tensor(
                                t, ypsum[(dr, dtc)], 1.0, zg[dtc],
                                OP.mult, OP.mult,
                                accum_out=m_sb[:, c : c + 1],
                            )
                            nc.gpsimd.dma_start(
                                out=bass.AP(tensor=u_cc_in, offset=128 * c,
                                            ap=[[1, 128]]),
                                in_=m_sb[:, c : c + 1],
                            )

        # ---- exchange pooled vector (fp32): u2[p, j] = v_full[p + 128j] ----
        with tc.tile_pool(name="ps_tail", bufs=1, space="PSUM") as ps_tail:
            nc.gpsimd.collective_compute(
                "AllGather", OP.bypass,
                replica_groups=[[0, 1], [2, 3], [4, 5], [6, 7]],
                ins=[u_cc_in[:, :]], outs=[u_cc_out[:, :]],
            )
            # prefetch the Sigmoid table while the collective runs; reading
            # m_sb pins the op after the accumulates (the scheduler would
            # otherwise float a dependency-free op anywhere)
            nc.scalar.activation(tdum, m_sb[0:1, 0:4], AF.Sigmoid)
            # fetch the gathered vector in two halves so the gate matvec's
            # first four columns start ~1us before the second half lands
            u2 = apool.tile([128, 8], F32, tag="u2", name="u2")
            u2b = apool.tile([128, 8], BF16, tag="u2b", name="u2b")
            for hh in range(2):
                nc.sync.dma_start(
                    out=u2[:, 4 * hh : 4 * hh + 4],
                    in_=bass.AP(tensor=u_cc_out, offset=512 * hh,
                                ap=[[1, 128], [128, 4]]),
                )
                nc.scalar.activation(
                    u2b[:, 4 * hh : 4 * hh + 4], u2[:, 4 * hh : 4 * hh + 4],
                    AF.Copy,
                )

            # ---- gate: q = G2^T v + b (bias via ones-row matmul); sigmoid ----
            vps = ps_tail.tile([1, 2 * DH], F32, tag="vps", name="vps")
            nc.tensor.matmul(vps, ones11, bgate_r, start=True, stop=False)
            for kc in range(8):
                nc.tensor.matmul(
                    vps, u2b[:, kc : kc + 1], g2[kc], start=False, stop=(kc == 7)
                )
            g_row = apool.tile([1, 2 * DH], F32, tag="grow", name="grow")
            nc.scalar.activation(g_row, vps, AF.Sigmoid)
            # reshape [1,512] -> [128,4] per-partition scalars: four
            # [1,128]->[128,1] SBUF-to-SBUF column DMAs (3-dim APs balance;
            # the single 4-dim reshape DMA does not, and a PE-transpose
            # variant intermittently produced NaN / wedged the device)
            g4 = apool.tile([128, 4], F32, tag="g4", name="g4")
            for j in range(4):
                nc.sync.dma_start(
                    out=g4[:, j : j + 1],
                    in_=g_row[0:1, j * 128 : (j + 1) * 128],
                )

            # fold the gate into the out-proj weight rows (channel-parallel)
            wos = [apool.tile([128, DM], BF16, tag=f"wos{i}", name=f"wos{i}")
                   for i in range(4)]
            for kc in range(4):
                nc.vector.tensor_scalar_mul(wos[kc], wo[kc], g4[:, kc : kc + 1])

            out_sb = [apool.tile([128, L], F32, tag=f"o{i}", name=f"o{i}")
                      for i in range(2)]
            for pc in range(2):
                ops_ = ps_tail.tile([128, L], F32, tag="ops", name="ops")
                for f0, fl in FCH:
                    for kc in range(4):
                        nc.tensor.matmul(
                            ops_[:, f0 : f0 + fl],
                            wos[kc][:, pc * 128 : (pc + 1) * 128],
                            yg[(kc // 2, kc % 2)][:, f0 : f0 + fl],
                            start=(kc == 0), stop=(kc == 3),
                        )
                    # copy + store per chunk so the last DMA tail is short
                    nc.scalar.activation(
                        out_sb[pc][:, f0 : f0 + fl], ops_[:, f0 : f0 + fl], AF.Copy
                    )
                    nc.sync.dma_start(
                        out=outp[pc * 128 : (pc + 1) * 128, f0 : f0 + fl],
                        in_=out_sb[pc][:, f0 : f0 + fl],
                    )

    nc.finalize()
    return nc


_NC_CACHE = {}


def _get_module(shared_a: bool):
    if shared_a not in _NC_CACHE:
        _NC_CACHE[shared_a] = _build_module(shared_a)
    return _NC_CACHE[shared_a]


def _diag_stack(d):
    out = np.zeros((DH, 128), dtype=np.float32)
    for t in range(NDT):
        out[t * 128 : (t + 1) * 128, :] = np.diag(d[t * 128 : (t + 1) * 128])
    return out


def kernel(**inputs):
    inp = {k: np.asarray(v, dtype=np.float32) for k, v in inputs.items()}
    hs = inp["hidden_states"]
    W_in, W_x, W_dt = inp["W_in"], inp["W_xproj"], inp["W_dt"]
    b_dt = inp["b_dt"]
    A_f = -np.exp(inp["A_log_f"])
    A_b = -np.exp(inp["A_log_b"])
    D_f, D_b = inp["D_f"], inp["D_b"]
    W_g, b_g = inp["W_global"], inp["b_global"]
    W_gate, b_gate = inp["W_gate"], inp["b_gate"]
    W_out = inp["W_out"]

    shared_a = bool(np.array_equal(A_f, A_b))
    I = np.eye(128, dtype=np.float32)
    bf = ml_dtypes.bfloat16
    in_maps = []
    for core in range(8):
        b, h = core // 2, core % 2
        o = h * DH
        perm = np.r_[o : o + DH, (DH - o) % DI : (DH - o) % DI + DH]
        ownc = np.r_[o : o + DH, DI + o : DI + o + DH]
        ccorder = np.r_[0:DH, DI : DI + DH, DH:DI, DI + DH : 2 * DI]

        def acol(A):
            a = A[o : o + DH].reshape(NDT, 128, DS)
            return np.ascontiguousarray(a.transpose(1, 0, 2).reshape(128, NDT * DS))

        m = {
            "hsT": np.ascontiguousarray(hs[b].T).astype(bf),
            "WinxT": np.ascontiguousarray(W_in[:DI][perm].T).astype(bf),
            "WinzT": np.ascontiguousarray(W_in[DI + o : DI + o + DH].T).astype(bf),
            "WxT": np.ascontiguousarray(W_x[:, perm].T).astype(bf),
            "WdtT": np.ascontiguousarray(W_dt[o : o + DH].T).astype(bf),
            "bdt": np.ascontiguousarray(b_dt[o : o + DH].reshape(NDT, 128).T),
            "Afc": acol(A_f),
            "Abc": acol(A_b),
            "Ddf": _diag_stack(D_f[o : o + DH]).astype(bf),
            "Ddb": _diag_stack(D_b[o : o + DH]).astype(bf),
            "I128": I.astype(bf),
            "G2T": np.ascontiguousarray(
                (W_gate[ownc] @ W_g[:, ccorder] / np.float32(L)).T
            ).astype(bf),
            "bgate2": np.ascontiguousarray(
                (b_gate[ownc] + W_gate[ownc] @ b_g).reshape(1, 512)
            ).astype(bf),
            "WoT": np.ascontiguousarray(W_out[:, ownc].T).astype(bf),
        }
        in_maps.append(m)

    nc = _get_module(shared_a)
    res = run_bass_kernel_spmd(nc, in_maps, core_ids=list(range(8)))
    outs = res.results
    out = np.zeros((B, L, DM), dtype=np.float32)
    for b in range(B):
        part = outs[2 * b]["outp"] + outs[2 * b + 1]["outp"]
        out[b] = part.T
    return out
